# revision 1
# baseline (speedup 1.0000x reference)
"""AdaptiveSpanAttention Trainium2 kernel (8 NeuronCores).

Sharding: core c -> (batch b = c//2, head-group g = c%2).
Each core computes, for its batch and its 8 heads:
  Q/K/V projections, anti-causal (j>=i) attention with adaptive-span
  mask, renormalization, and a partial output projection
  y_part = Out_g @ Wo[:, e_slice].T  (contraction over its 512 channels).
Host combines: y[b] = y_part[2b] + y_part[2b+1] + bo.

All matmuls in bf16 (f32 PSUM accumulation). Span-mask ramp in fp16
(exact for integer distances). No collectives.

Schedule notes:
- host packs x / weights into mega-tiles (xT8 [128, T, dtile],
  w8 [128, E, dtile]) so each tensor is 1-4 large DMAs; the HWDGE
  issue pipe (~0.6us per DMA) bounds the lead-in otherwise.
- attention is software-pipelined: scores for block st+1 are issued
  before attn@V of block st so the exp+mask chain hides under PE work.
- span bounds specialized per call from host-computed z: mask ops only
  on the ramp band, fully-masked score columns skipped.
"""
import sys

sys.path.insert(0, "/opt/trn_rl_repo")

from contextlib import ExitStack

import ml_dtypes
import numpy as np

import concourse.bass as bass
import concourse.tile as tile
from concourse import bacc, mybir
from concourse.bass_utils import run_bass_kernel_spmd

BF16 = mybir.dt.bfloat16
F16 = mybir.dt.float16
F32 = mybir.dt.float32

B, T, D, H = 4, 1024, 1024, 16
DH = 64          # head dim
R = 256.0
HC = 8           # heads per core
E = 512          # channels per core (HC * DH)
N_CORES = 8
TCH = 512        # t-chunk width (PSUM f32 free-dim limit)
NT = T // TCH    # 2 t-chunks
ST = T // 128    # 8 s-tiles
DT = D // 128    # 8 d-tiles

_NC_CACHE = {}


def causal_width(st, tch):
    """Valid query-column width of block (s_tile=st, t_chunk=tch)."""
    delta = 128 * st - 512 * tch
    return max(0, min(TCH, delta + 128))


def span_width(st, tch, zlo):
    """Columns [0, m_w) where the span mask can differ from 1 (z >= zlo)."""
    delta = 128 * st - 512 * tch
    w = causal_width(st, tch)
    return max(0, min(w, delta + 127 - int(zlo)))


def dead_width(st, tch, zhi):
    """Columns [0, d_w) where the span mask is identically 0 (z <= zhi)."""
    delta = 128 * st - 512 * tch
    w = causal_width(st, tch)
    return max(0, min(w, int(delta - R - zhi) // 16 * 16))


def build_nc(zlo, zhi, skip384):
    key = (zlo, zhi, skip384)
    if key in _NC_CACHE:
        return _NC_CACHE[key]
    nc = bacc.Bacc("TRN2", target_bir_lowering=False, debug=False, num_devices=1)

    # ---- DRAM parameters (per-core shards prepared on host) ----
    xT_d = nc.declare_dram_parameter("xT8", [128, T, DT], BF16, isOutput=False)
    Wq_d = nc.declare_dram_parameter("Wq8", [128, E, DT], BF16, isOutput=False)
    Wk_d = nc.declare_dram_parameter("Wk8", [128, E, DT], BF16, isOutput=False)
    Wv_d = nc.declare_dram_parameter("Wv8", [128, E, DT], BF16, isOutput=False)
    Wo_d = nc.declare_dram_parameter("Wo8", [128, D, 4], BF16, isOutput=False)
    Wsp_d = nc.declare_dram_parameter("Wsp8", [128, HC, DT], BF16, isOutput=False)
    bspan_d = nc.declare_dram_parameter("bspan", [1, HC], F32, isOutput=False)
    # packed span-ramp tiles: for each k with nonzero span width m_k, columns
    # [off_k, off_k+m_k) hold cneg[k, s', t'] = -(128k + s' - t')/R
    # (-60000 where causal-invalid)
    widths = [span_width(st, 0, zlo) for st in range(ST)]
    offs = np.concatenate([[0], np.cumsum(widths)]).astype(int)
    cneg_d = nc.declare_dram_parameter("cneg", [128, max(1, int(offs[-1]))],
                                       F16, isOutput=False)
    # c01[s', k, j] = 1.0 if s' >= j else 0.0  (causal 0/1 for t' = 128k + j)
    c01_d = nc.declare_dram_parameter("c01", [128, 4, 128], F16, isOutput=False)
    yp_d = nc.declare_dram_parameter("yp", [T, D], F16, isOutput=True)

    with tile.TileContext(nc) as tc, ExitStack() as ctx:
        # ---------------- pools ----------------
        consts = ctx.enter_context(tc.tile_pool(name="consts", bufs=1))
        xp = ctx.enter_context(tc.tile_pool(name="xp", bufs=1))
        wp = ctx.enter_context(tc.tile_pool(name="wp", bufs=1))
        qkp = ctx.enter_context(tc.tile_pool(name="qkp", bufs=1))
        vp = ctx.enter_context(tc.tile_pool(name="vp", bufs=1))
        outp = ctx.enter_context(tc.tile_pool(name="outp", bufs=1))
        scr = ctx.enter_context(tc.tile_pool(name="scr", bufs=3))
        ysb = ctx.enter_context(tc.tile_pool(name="ysb", bufs=6))

        lead_ctx = ExitStack()
        ps_lead = lead_ctx.enter_context(
            tc.tile_pool(name="ps_lead", bufs=8, space="PSUM"))

        ones_row = consts.tile([1, 128], BF16)
        nc.vector.memset(ones_row[:], 1.0)

        # ---------------- loads (few large DMAs) ----------------
        xT = xp.tile([128, T, DT], BF16, name="xT8")
        wq = wp.tile([128, E, DT], BF16, name="wq8")
        wk = wp.tile([128, E, DT], BF16, name="wk8")
        wv = wp.tile([128, E, DT], BF16, name="wv8")
        wsp = wp.tile([128, HC, DT], BF16, name="wsp8")
        wo = wp.tile([128, D, 4], BF16, name="wo8")
        # wq[256:] feeds only the attention-phase fillers, so it streams
        # after wv; xT is sliced so each Q group's data lands just in time
        nc.sync.dma_start(xT[:, 0:64, :], xT_d[:, 0:64, :])
        nc.sync.dma_start(wq[:, 0:128, :], Wq_d[:, 0:128, :])
        nc.sync.dma_start(xT[:, 64:128, :], xT_d[:, 64:128, :])
        nc.sync.dma_start(xT[:, 128:256, :], xT_d[:, 128:256, :])
        nc.sync.dma_start(wq[:, 128:256, :], Wq_d[:, 128:256, :])
        nc.sync.dma_start(xT[:, 256:512, :], xT_d[:, 256:512, :])
        nc.sync.dma_start(xT[:, 512:768, :], xT_d[:, 512:768, :])
        nc.sync.dma_start(xT[:, 768:1024, :], xT_d[:, 768:1024, :])
        nc.sync.dma_start(wsp[:], Wsp_d[:, :, :])
        nc.sync.dma_start(wk[:], Wk_d[:, :, :])
        nc.sync.dma_start(wv[:], Wv_d[:, :, :])
        bspan_sb = consts.tile([1, HC], F32)
        nc.sync.dma_start(bspan_sb[:], bspan_d[:, :])
        cneg_sb = consts.tile([128, max(1, int(offs[-1]))], F16, tag="cneg")
        nc.sync.dma_start(cneg_sb[:], cneg_d[:, :])
        c01_sb = consts.tile([128, 4, 128], F16, tag="c01")
        nc.sync.dma_start(c01_sb[:], c01_d[:, :, :])
        nc.sync.dma_start(wq[:, 256:E, :], Wq_d[:, 256:E, :])
        nc.sync.dma_start(wo[:], Wo_d[:, :, :])

        # ---------------- Q/K projections (transposed layout) ----------------
        # QT[e, t] = sum_d W[d, e] * xT[d, t]; psum -> bf16 copies
        qt_sb = [qkp.tile([128, T], BF16, tag="qt", name=f"qt{i}", bufs=4)
                 for i in range(4)]
        kt_sb = [qkp.tile([128, T], BF16, tag="kt", name=f"kt{i}", bufs=4)
                 for i in range(4)]

        def emit_proj(dst, w8, et, t0, t1, copy_eng="act", pool=None):
            pool = pool or ps_lead
            ps = pool.tile([128, TCH], F32, tag="pj", name=f"pj{et}_{t0}",
                           padded_shape=[128, TCH])
            for n_i in range(DT):
                nc.tensor.matmul(
                    ps[:, 0:t1 - t0],
                    w8[:, 128 * et:128 * (et + 1), n_i],
                    xT[:, t0:t1, n_i],
                    start=(n_i == 0), stop=(n_i == DT - 1))
            if copy_eng == "act":
                nc.scalar.copy(dst[et][:, t0:t1], ps[:, 0:t1 - t0])
            elif copy_eng == "pool":
                nc.vector.tensor_copy(dst[et][:, t0:t1], ps[:, 0:t1 - t0])
            else:
                nc.vector.tensor_copy(dst[et][:, t0:t1], ps[:, 0:t1 - t0])

        # first groups sized/ordered to the sliced lead DMA stream
        emit_proj(qt_sb, wq, 0, 0, 64)
        emit_proj(qt_sb, wq, 0, 64, 128)
        emit_proj(qt_sb, wq, 0, 128, 256)
        emit_proj(qt_sb, wq, 1, 0, 256)
        emit_proj(qt_sb, wq, 0, 256, 512)
        emit_proj(qt_sb, wq, 1, 256, 512)
        emit_proj(qt_sb, wq, 0, 512, 768)
        emit_proj(qt_sb, wq, 1, 512, 768)
        emit_proj(qt_sb, wq, 0, 768, 1024)
        emit_proj(qt_sb, wq, 1, 768, 1024)

        # span-net partial sums (xT fully resident by now); the serial
        # DVE/ACT sigmoid chain overlaps the K/V projection matmuls
        msum = consts.tile([128, DT], BF16)
        with nc.allow_low_precision(reason="span-net mean in bf16 is plenty"):
            for dt_i in range(DT):
                nc.vector.tensor_reduce(
                    msum[:, dt_i:dt_i + 1], xT[:, :, dt_i],
                    mybir.AxisListType.X, mybir.AluOpType.add)

        for et in (0, 1):
            for tch in range(NT):
                emit_proj(kt_sb, wk, et, TCH * tch, TCH * (tch + 1))

        # ---------------- V (natural layout, ones-augmented) ----------------
        # v_aug[st][p, h, 0:64] = V[128*st+p, 64h+j]; v_aug[st][p, h, 64:128] = 1
        # (64 ones columns make attn@V produce the denominator W broadcast
        #  across psum partitions 64:128)
        v_aug = [None] * ST

        def emit_v(st, pool=None, copy_eng="dve"):
            pool = pool or ps_lead
            va = vp.tile([128, HC, 2 * DH], BF16, tag="vaug", bufs=ST,
                         name=f"vaug{st}")
            nc.gpsimd.memset(va[:, :, DH:2 * DH], 1.0)
            ps = pool.tile([128, E], F32, tag="pj", name=f"pjv{st}")
            for n_i in range(DT):
                nc.tensor.matmul(
                    ps[:],
                    xT[:, 128 * st:128 * (st + 1), n_i],
                    wv[:, :, n_i],
                    start=(n_i == 0), stop=(n_i == DT - 1))
            ceng = nc.scalar.copy if copy_eng == "act" else nc.vector.tensor_copy
            ceng(va[:, :, 0:DH], ps[:].rearrange("p (h d) -> p h d", h=HC))
            v_aug[st] = va

        # ---------------- span net ----------------
        # logit = (sum_t x)/T @ WspanT + bspan; a = 1 + (T/R)*sigmoid(logit)
        # PE part (zlog) is tiny; the DVE/ACT sigmoid chain overlaps the V
        # projection matmuls below.
        zlog = ps_lead.tile([1, HC], F32, tag="pj", padded_shape=[128, TCH])
        for dt_i in range(DT):
            nc.tensor.matmul(zlog[:], msum[:, dt_i:dt_i + 1],
                             wsp[:, :, dt_i],
                             start=(dt_i == 0), stop=(dt_i == DT - 1))
        zrow = consts.tile([1, HC], F32)
        nc.vector.scalar_tensor_tensor(
            zrow[:], zlog[:], 1.0 / T, bspan_sb[:],
            op0=mybir.AluOpType.mult, op1=mybir.AluOpType.add)
        # sigmoid via exp so ACT stays on the exp LUT table mid-stream
        en = consts.tile([1, HC], F32)
        nc.scalar.activation(en[:], zrow[:],
                             mybir.ActivationFunctionType.Exp, scale=-1.0)
        den = consts.tile([1, HC], F32)
        nc.vector.tensor_scalar_add(den[:], en[:], 1.0)
        sig = consts.tile([1, HC], BF16)
        with nc.allow_low_precision(reason="span sigmoid recip in bf16"):
            nc.vector.reciprocal(sig[:], den[:])

        emit_v(0)
        emit_v(1)
        a_ps = ps_lead.tile([128, HC], F32, tag="pj", padded_shape=[128, TCH])
        nc.tensor.matmul(a_ps[:], ones_row[:], sig[:], start=True, stop=True)
        a_sb = consts.tile([128, HC], F32)
        nc.scalar.activation(a_sb[:], a_ps[:],
                             mybir.ActivationFunctionType.Identity,
                             scale=T / R, bias=1.0)
        emit_v(2, copy_eng="act")
        emit_v(3, copy_eng="act")

        # precompute span-mask ramps mt[h,k] = max(a_h + cneg_k, 0) on DVE
        # during the lead (independent of attention p); per-block mask then
        # needs only one min+mult op, shortening the exp->attn@V chain
        mt_pre = {}
        mt_cols = sum(
            max(0, span_width(k, 0, zlo) - dead_width(k, 0, zhi))
            for k in range(ST) if not (k == 3 and skip384))
        for k in range(ST if mt_cols <= 2600 else 0):
            m_w = span_width(k, 0, zlo)
            if m_w <= 0 or (k == 3 and skip384):
                continue
            d_w = dead_width(k, 0, zhi)
            for h in range(HC):
                mt = consts.tile([128, m_w - d_w], F16, tag=f"mtp{k}",
                                 bufs=HC, name=f"mtp{k}_{h}")
                nc.vector.tensor_scalar(
                    mt[:],
                    cneg_sb[:, offs[k] + d_w:offs[k] + m_w],
                    a_sb[:, h:h + 1], 0.0,
                    op0=mybir.AluOpType.add, op1=mybir.AluOpType.max)
                mt_pre[(h, k)] = (mt, d_w)

        # one spare projection group at the lead tail keeps PE busy while
        # the first score block's psum bank clears its lead-phase WAR
        emit_proj(qt_sb, wq, 2, 0, TCH)

        lead_ctx.close()
        attn_ctx = ExitStack()
        ps_sc = attn_ctx.enter_context(
            tc.tile_pool(name="ps_sc", bufs=2, space="PSUM"))
        ps_out = attn_ctx.enter_context(
            tc.tile_pool(name="ps_out", bufs=2, space="PSUM"))
        ps_fill = attn_ctx.enter_context(
            tc.tile_pool(name="ps_fill", bufs=2, space="PSUM"))

        def proj_fillers(specs):
            """Filler closures whose psum->SBUF copy is deferred one slot so
            it queues behind the current block's exp/mask, not ahead."""
            fs = []
            pend = [None]

            def make(dst, w8, et, tch):
                def f():
                    ps = ps_fill.tile([128, TCH], F32, tag="pj",
                                      name=f"pjf{et}_{tch}",
                                      padded_shape=[128, TCH])
                    for n_i in range(DT):
                        nc.tensor.matmul(
                            ps[:],
                            w8[:, 128 * et:128 * (et + 1), n_i],
                            xT[:, TCH * tch:TCH * (tch + 1), n_i],
                            start=(n_i == 0), stop=(n_i == DT - 1))
                    prev, pend[0] = pend[0], (
                        lambda: nc.vector.tensor_copy(
                            dst[et][:, TCH * tch:TCH * (tch + 1)], ps[:]))
                    if prev is not None:
                        prev()
                return f

            for dst, w8, et, tch in specs:
                fs.append(make(dst, w8, et, tch))

            def flush():
                if pend[0] is not None:
                    pend[0]()
                    pend[0] = None
            fs.append(flush)
            return fs

        # ---------------- attention ----------------
        # out_pair[j][tch] holds heads 2j (parts 0:64) and 2j+1 (parts 64:128)
        out_pair = [[outp.tile([128, TCH], BF16, tag="out", bufs=8,
                               name=f"op{j}_{c}") for c in range(NT)]
                    for j in range(4)]

        def attn_pair(tch, j, v_prefetch=False, fillers=(), prev_fin=None,
                      split_fin=False, split_exp=False):
            """Attention for head pair (2j, 2j+1); both share et=j.

            Scores for the two heads go into one 2-bank psum pair-tile so a
            single exp covers both. Scores run one block ahead of attn@V so
            the exp+mask chain hides under PE work. The previous pair's
            out-division (prev_fin) is emitted after this pair's first score
            block so it does not wedge ahead of this pair's mask ops in the
            DVE queue. Returns this pair's finalize closure.
            """
            first_st = 4 * tch
            heads = (2 * j, 2 * j + 1)
            pouts = [ps_out.tile([128, TCH], F32, tag="pout",
                                 name=f"pout{h}_{tch}") for h in heads]
            fillers = list(fillers)
            p_tiles = {}

            def block_ranges(st):
                w = causal_width(st, tch)
                d_w = dead_width(st, tch, zhi)
                return [(d_w, w)]

            def emit_sc(st):
                w = causal_width(st, tch)
                m_w = span_width(st, tch, zlo)
                d_w = dead_width(st, tch, zhi)
                k = st - first_st  # delta = 128*k
                sc_hp = ps_sc.tile([128, 2, TCH], F32, tag="sc",
                                   name=f"sc{j}_{st}")
                p_hp = scr.tile([128, 2, TCH], BF16, tag="p", bufs=10,
                                name=f"p{j}_{st}")
                for c0, c1 in block_ranges(st):
                    for i, h in enumerate(heads):
                        hp = (h % 2) * 64
                        nc.tensor.matmul(
                            sc_hp[:, i, c0:c1],
                            kt_sb[j][hp:hp + DH, 128 * st:128 * (st + 1)],
                            qt_sb[j][hp:hp + DH,
                                     TCH * tch + c0:TCH * tch + c1],
                            start=True, stop=True)
                    nc.scalar.activation(
                        p_hp[:, :, c0:c1], sc_hp[:, :, c0:c1],
                        mybir.ActivationFunctionType.Exp, scale=1.0 / 8.0)
                    for i, h in enumerate(heads):
                        if k <= 3:
                            # diagonal block: causal zeroing on [128k, w)
                            d0 = 128 * k
                            v0, v1 = max(c0, d0), min(c1, w)
                            if v1 > v0:
                                nc.gpsimd.tensor_mul(
                                    p_hp[:, i, v0:v1], p_hp[:, i, v0:v1],
                                    c01_sb[:, k, v0 - d0:v1 - d0])
                        if (h, k) in mt_pre:
                            # span mask: pm = min(mt, 1) * p (precomputed mt)
                            mt, mt_d = mt_pre[(h, k)]
                            v0, v1 = max(c0, d_w), min(c1, m_w)
                            if v1 > v0:
                                nc.vector.scalar_tensor_tensor(
                                    p_hp[:, i, v0:v1],
                                    mt[:, v0 - mt_d:v1 - mt_d], 1.0,
                                    p_hp[:, i, v0:v1],
                                    op0=mybir.AluOpType.min,
                                    op1=mybir.AluOpType.mult)
                        elif m_w > d_w and not (k == 3 and skip384):
                            # fallback: inline ramp when mt_pre was skipped
                            v0, v1 = max(c0, d_w), min(c1, m_w)
                            if v1 > v0:
                                mt = scr.tile([128, TCH], F16, tag="mtf",
                                              bufs=8, name=f"mtf{h}_{st}")
                                nc.vector.tensor_scalar(
                                    mt[:, v0:v1],
                                    cneg_sb[:, offs[k] + v0:offs[k] + v1],
                                    a_sb[:, h:h + 1], 0.0,
                                    op0=mybir.AluOpType.add,
                                    op1=mybir.AluOpType.max)
                                nc.vector.scalar_tensor_tensor(
                                    p_hp[:, i, v0:v1], mt[:, v0:v1], 1.0,
                                    p_hp[:, i, v0:v1],
                                    op0=mybir.AluOpType.min,
                                    op1=mybir.AluOpType.mult)
                p_tiles[st] = p_hp

            def emit_av(st):
                av_first = ST - 1 if tch == 1 else first_st
                av_last = first_st if tch == 1 else ST - 1
                for c0, c1 in block_ranges(st):
                    for i, h in enumerate(heads):
                        nc.tensor.matmul(
                            pouts[i][:, c0:c1], v_aug[st][:, h, :],
                            p_tiles[st][:, i, c0:c1],
                            start=(st == av_first), stop=(st == av_last),
                            skip_group_check=True)

            # tch=1 pairs run largest-first: the pair ends on its smallest
            # exp, so the next pair's score banks are freed ~3x sooner at
            # the handoff (tch=0 keeps ascending; v-prefetch requires it and
            # its wide late blocks have no filler cover early)
            desc = tch == 1
            order = (list(range(ST - 1, first_st - 1, -1)) if desc
                     else list(range(first_st, ST)))
            emit_sc(order[0])
            if prev_fin is not None:
                prev_fin()
            for idx in range(1, len(order)):
                st = order[idx]
                if v_prefetch and st + 3 < ST and v_aug[st + 3] is None:
                    emit_v(st + 3, pool=ps_fill)
                if fillers:
                    fillers.pop(0)()
                emit_sc(st)
                if idx >= 2:
                    emit_av(order[idx - 2])
            if fillers:
                fillers.pop(0)()
            emit_av(order[-2])
            emit_av(order[-1])

            def finalize():
                # rows 0:64 numerator; rows 64:128 denominator W (broadcast)
                rws = []
                for i, h in enumerate(heads):
                    rw = scr.tile([DH, TCH], F32, tag="rw", bufs=8,
                                  name=f"rw{h}")
                    with nc.allow_low_precision(reason="denom recip bf16"):
                        nc.vector.reciprocal(rw[:], pouts[i][DH:2 * DH, :])
                    rws.append(rw)
                # column-chunked division (split_fin) lets tail y-groups for
                # early tt unblock before the full 512 columns are divided
                chunks = (0, 256, TCH) if split_fin else (0, TCH)
                for c0, c1 in zip(chunks[:-1], chunks[1:]):
                    for i, h in enumerate(heads):
                        hp = (h % 2) * 64
                        nc.vector.scalar_tensor_tensor(
                            out_pair[j][tch][hp:hp + DH, c0:c1],
                            pouts[i][0:DH, c0:c1], 1.0, rws[i][:, c0:c1],
                            op0=mybir.AluOpType.mult,
                            op1=mybir.AluOpType.mult)

            return finalize

        copy_rot = [0]

        def y_group(tt, nch, pool, engines=("act", "pool"), yo=None):
            """One output-projection psum group; DMA fires unless yo is a
            shared per-tt tile whose DMA the caller batches."""
            tch = tt // 4
            toff = 128 * tt - TCH * tch
            yps = pool.tile([128, TCH], F32, tag="pj", name=f"y{tt}_{nch}")
            for j in range(4):
                nc.tensor.matmul(
                    yps[:],
                    out_pair[j][tch][:, toff:toff + 128],
                    wo[:, TCH * nch:TCH * (nch + 1), j],
                    start=(j == 0), stop=(j == 3))
            own_dma = yo is None
            if own_dma:
                yo_sl = ysb.tile([128, TCH], F16, tag="y")
            else:
                yo_sl = yo[:, TCH * nch:TCH * (nch + 1)]
            eng = engines[copy_rot[0] % len(engines)]
            copy_rot[0] += 1
            if eng == "dve":
                nc.vector.tensor_copy(yo_sl[:] if own_dma else yo_sl, yps[:])
            else:
                nc.scalar.copy(yo_sl[:] if own_dma else yo_sl, yps[:])
            if own_dma:
                nc.sync.dma_start(
                    yp_d[128 * tt:128 * (tt + 1), TCH * nch:TCH * (nch + 1)],
                    yo_sl[:])

        def y_fillers(tts):
            fs = []
            pend = [None]

            def make(tt, nch):
                def f():
                    tch0 = tt // 4
                    toff = 128 * tt - TCH * tch0
                    yps = ps_fill.tile([128, TCH], F32, tag="pj",
                                       name=f"y{tt}_{nch}")
                    for j in range(4):
                        nc.tensor.matmul(
                            yps[:],
                            out_pair[j][tch0][:, toff:toff + 128],
                            wo[:, TCH * nch:TCH * (nch + 1), j],
                            start=(j == 0), stop=(j == 3))

                    def copy_dma():
                        yo = ysb.tile([128, TCH], F16, tag="y")
                        nc.vector.tensor_copy(yo[:], yps[:])
                        nc.sync.dma_start(
                            yp_d[128 * tt:128 * (tt + 1),
                                 TCH * nch:TCH * (nch + 1)], yo[:])
                    prev, pend[0] = pend[0], copy_dma
                    if prev is not None:
                        prev()
                return f

            for tt in tts:
                for nch in range(NT):
                    fs.append(make(tt, nch))

            def flush():
                if pend[0] is not None:
                    pend[0]()
                    pend[0] = None
            fs.append(flush)
            return fs

        f1 = proj_fillers([(qt_sb, wq, 2, 1), (kt_sb, wk, 2, 0),
                           (kt_sb, wk, 2, 1)])
        f2 = proj_fillers([(qt_sb, wq, 3, 0), (qt_sb, wq, 3, 1),
                           (kt_sb, wk, 3, 0)])
        f3 = proj_fillers([(kt_sb, wk, 3, 1)])
        fin = attn_pair(0, 0, v_prefetch=True)
        fin = attn_pair(0, 1, fillers=f1, prev_fin=fin)
        fin = attn_pair(0, 2, fillers=f2, prev_fin=fin)
        fin = attn_pair(0, 3, fillers=f3, prev_fin=fin)
        fin = attn_pair(1, 0, fillers=y_fillers([0]), prev_fin=fin)
        fin = attn_pair(1, 1, fillers=y_fillers([1]), prev_fin=fin)
        fin = attn_pair(1, 2, fillers=y_fillers([2]), prev_fin=fin)
        fin = attn_pair(1, 3, fillers=y_fillers([3]), prev_fin=fin,
                        split_fin=True)
        fin()
        attn_ctx.close()
        with tc.tile_pool(name="ps_tail", bufs=4, space="PSUM") as ps_tail:
            # tts 4-6 batch both 512-col chunks into one DMA; the last tt
            # keeps per-chunk DMAs so its final chain is short
            for tt in range(4, 7):
                yo = ysb.tile([128, D], F16, tag="yb", bufs=3)
                for nch in range(NT):
                    y_group(tt, nch, ps_tail,
                            engines=("dve", "act"), yo=yo)
                nc.sync.dma_start(yp_d[128 * tt:128 * (tt + 1), :], yo[:])
            for nch in range(NT):
                y_group(7, nch, ps_tail, engines=("act", "dve"))

    nc.compile()
    _NC_CACHE[key] = nc
    return nc


def _pack_dtiles(w):
    """[D, F] -> [128, F, DT] mega-tile (partition, inner, d-tile)."""
    Dd, F = w.shape
    return np.ascontiguousarray(
        w.reshape(Dd // 128, 128, F).transpose(1, 2, 0))


def _prep_core_inputs(x, Wq, Wk, Wv, Wo, Wspan, bspan, cneg, c01):
    bf = ml_dtypes.bfloat16
    in_maps = []
    for c in range(N_CORES):
        b, g = c // 2, c % 2
        hs = slice(E * g, E * (g + 1))
        in_maps.append({
            "c01": c01,
            "xT8": _pack_dtiles(np.ascontiguousarray(x[b].T)).astype(bf),
            "Wq8": _pack_dtiles(np.ascontiguousarray(Wq[hs, :].T)).astype(bf),
            "Wk8": _pack_dtiles(np.ascontiguousarray(Wk[hs, :].T)).astype(bf),
            "Wv8": _pack_dtiles(np.ascontiguousarray(Wv[hs, :].T)).astype(bf),
            "Wo8": _pack_dtiles(np.ascontiguousarray(Wo[:, hs].T)).astype(bf),
            "Wsp8": _pack_dtiles(
                np.ascontiguousarray(Wspan[HC * g:HC * (g + 1), :].T)).astype(bf),
            "bspan": np.asarray(bspan[HC * g:HC * (g + 1)], np.float32).reshape(1, HC),
            "cneg": cneg,
        })
    return in_maps


def _make_c01():
    sp = np.arange(128, dtype=np.float32)[:, None]
    jp = np.arange(128, dtype=np.float32)[None, :]
    m = (sp - jp >= 0).astype(np.float16)
    return np.ascontiguousarray(
        np.broadcast_to(m[:, None, :], (128, 4, 128))).astype(np.float16)


def _make_cneg(zlo):
    sp = np.arange(128, dtype=np.float32)[:, None]
    cols = []
    for k in range(ST):
        m_w = span_width(k, 0, zlo)
        if m_w == 0:
            continue
        tp = np.arange(m_w, dtype=np.float32)[None, :]
        d = 128.0 * k + sp - tp
        ramp = np.where(d < 0, -60000.0, -d / R)
        cols.append(ramp)
    if not cols:
        return np.zeros((128, 1), np.float16)
    return np.concatenate(cols, axis=1).astype(np.float16)


def _nc_params(x, Wspan, bspan):
    """Span bounds from host-exact z; specializes mask widths per call."""
    x = np.asarray(x, np.float32)
    Wspan = np.asarray(Wspan, np.float32)
    bspan = np.asarray(bspan, np.float32)
    logits = x.mean(axis=1) @ Wspan.T + bspan
    z = T / (1.0 + np.exp(-logits))
    zlo = max(0, int(z.min() - 8.0) // 16 * 16)
    zhi = int(z.max() + 8.0) + 16
    # skipping the delta=384 ramp (dist in (z, 511]) perturbs <= (511-z)/R
    # of the weight on a sliver of columns; safe when z_min >= 491
    skip384 = bool(z.min() >= 491.0)
    return zlo, zhi, skip384


def kernel(x, Wq, Wk, Wv, Wo, bo, Wspan, bspan):
    x = np.asarray(x, np.float32)
    Wq = np.asarray(Wq, np.float32)
    Wk = np.asarray(Wk, np.float32)
    Wv = np.asarray(Wv, np.float32)
    Wo = np.asarray(Wo, np.float32)
    bo = np.asarray(bo, np.float32)
    Wspan = np.asarray(Wspan, np.float32)
    bspan = np.asarray(bspan, np.float32)

    zlo, zhi, skip384 = _nc_params(x, Wspan, bspan)
    nc = build_nc(zlo, zhi, skip384)
    in_maps = _prep_core_inputs(x, Wq, Wk, Wv, Wo, Wspan, bspan,
                                _make_cneg(zlo), _make_c01())
    res = run_bass_kernel_spmd(nc, in_maps, core_ids=list(range(N_CORES)))
    y = np.empty((B, T, D), np.float32)
    for b in range(B):
        y[b] = (res.results[2 * b]["yp"].astype(np.float32)
                + res.results[2 * b + 1]["yp"].astype(np.float32) + bo)
    return y

